# revision 37
# baseline (speedup 1.0000x reference)
"""Trainium2 Bass kernel for nn_AttnOnlyTransformer_55929064128766.

Reference model: B=4, S=2048, D=2048 (=vocab), DQK=128, L=4 layers.
  enc0 = one_hot(token_ids, D) + sinusoidal_PE(S, D)
  per layer: q = enc@Wq; k = enc@Wk; A = softmax(mask(q k^T / sqrt(DQK)));
             enc = A @ enc
  output: enc  [B, S, D] f32

Sharding (8 cores): data-parallel over batch (4 pairs) x column-parallel
over D within each pair (DC = 1024 columns of enc per core).  Scores are
replicated within a pair; A @ enc splits by columns so the sharding is
preserved across layers.  q/k contract over all of D, so each core
computes a partial and the pair combines them with a bf16 AllGather +
on-device add (groups [0,1],[2,3],[4,5],[6,7]).

Numerics: fp8 e4m3 everywhere the 2e-2 tolerance allows (measured
~7e-3 end to end in simulation): enc (V operand + transposes), exp'd
scores, W (pre-scaled x16 to avoid fp8 subnormals; folded back in the
exp scale), with fp8 DoubleRow matmuls (2 key-tiles contracted per
instruction).  PSUM accumulation stays f32; the q/k exchange is bf16.

Per layer l:
  S1: land the two AllGather halves (issued during layer l-1) into
      qkT = [qT | kT] (dqk on partitions); scoresT_t = kT_t^T qT
      (bf16), additive -1e9 causal mask on the diagonal block only,
      exp -> fp8 expT pair tiles (t even/odd interleaved as DoubleRow
      k-subtiles, with a zeroed pad block for the odd tile).
  S2 (i = 15..8): out_i = sum_tp expT_pair_tp^T @ enc8[2tp:2tp+2]
      via DoubleRow, plus a ones column pair for the softmax
      denominator; normalize with the reciprocal on evacuation,
      writing the fp8 enc ping-pong buffer (and f32 staging -> DRAM on
      the last layer).
  phase A(l+1): PE-transpose enc8' rows 1024..2047 (fp8 transposes,
      stride-2 PSUM out) into encT pair tiles; project
      qkT_partial[:, 1024:2048] = W^T @ encT via DoubleRow (already
      normalized, so no carry-through-attention needed); issue
      AllGather half 1.
  S2 (i = 7..0), then phase B(l+1) (rows 0..1023) and half 0.
"""

import math

import numpy as np

B, S, D, DQK, L = 4, 2048, 2048, 128, 4
SPLIT = 2                 # cores per batch (column split factor)
DC = D // SPLIT           # columns of enc owned by one core
N_CORES = B * SPLIT
NT = S // 128             # number of 128-row tiles of the sequence (16)
NDT = DC // 128           # number of 128-col d-tiles per core (8)
NP = NT // 2              # number of key-tile pairs (8)
WSCALE = 16.0             # host pre-scale on W to keep fp8 normal
SCALE = 1.0 / math.sqrt(DQK) / (WSCALE * WSCALE)
GROUPS = [[2 * i, 2 * i + 1] for i in range(B)]

_CACHED = {}


def _build(reps=1, skip_cc=False, no_ag=False, dbg=False, no_dn=False,
           no_phase=False, no_dr=False, one_ag=False):
    import concourse.bass as bass  # noqa: F401
    import concourse.mybir as mybir
    import concourse.tile as tile
    from concourse import bacc

    F32 = mybir.dt.float32
    F32R = mybir.dt.float32r
    BF16 = mybir.dt.bfloat16
    FP8 = mybir.dt.float8e4
    DR = mybir.MatmulPerfMode.DoubleRow
    Exp = mybir.ActivationFunctionType.Exp
    Copy = mybir.ActivationFunctionType.Copy
    Add = mybir.AluOpType.add
    Mult = mybir.AluOpType.mult

    nc = bacc.Bacc("TRN2", target_bir_lowering=False, debug=False,
                   num_devices=N_CORES)

    # ---- I/O ----
    pe_nat = nc.dram_tensor("pe_nat", [S, DC], F32R, kind="ExternalInput").ap()
    tok_col = nc.dram_tensor("tok_col", [128, NT], F32, kind="ExternalInput").ap()
    iota_nat = nc.dram_tensor("iota_nat", [128, DC], F32, kind="ExternalInput").ap()
    w_stk_in = nc.dram_tensor("w_stk", [L, 128, NDT, 256], FP8,
                              kind="ExternalInput").ap()
    ident8_in = nc.dram_tensor("ident8", [128, 128], FP8, kind="ExternalInput").ap()
    utm_add_in = nc.dram_tensor("utm_add", [128, 128], F32, kind="ExternalInput").ap()
    out_dram = nc.dram_tensor("out", [S, DC], F32R, kind="ExternalOutput").ap()

    # two half-allgathers per layer; half h carries q/k columns for
    # queries [1024h, 1024h+1024): [qT cols | kT cols] (dqk on partitions);
    # fp8 payload halves the exchange bytes, Shared output speeds the CC
    cc_in_h = [nc.dram_tensor(f"cc_in{h}", [128, S], FP8, kind="Internal").ap()
               for h in range(2)]
    cc_out_h = [nc.dram_tensor(f"cc_out{h}", [SPLIT, 128, S], FP8,
                               kind="Internal").ap()
                for h in range(2)]
    # one_ag mode: a single per-layer collective carrying both halves
    cc_in_all = nc.dram_tensor("cc_in_all", [128, 2, S], FP8,
                               kind="Internal").ap()
    cc_out_all = nc.dram_tensor("cc_out_all", [SPLIT, 128, 2, S], FP8,
                                kind="Internal").ap()

    if dbg:
        dbg_enc8 = nc.dram_tensor("dbg_enc8", [128, NT, DC], FP8,
                                  kind="ExternalOutput").ap()
        dbg_carry = nc.dram_tensor("dbg_carry", [128, 2, S], BF16,
                                   kind="ExternalOutput").ap()
        dbg_qkT = nc.dram_tensor("dbg_qkT", [128, 2 * S], BF16,
                                 kind="ExternalOutput").ap()
        dbg_expT0 = nc.dram_tensor("dbg_expT0", [NP, 128, 2, S], FP8,
                                   kind="ExternalOutput").ap()
        dbg_dn = nc.dram_tensor("dbg_dn", [NT, 128, 1], F32,
                                kind="ExternalOutput").ap()
        dbg_enc1 = nc.dram_tensor("dbg_enc1", [128, NT, DC], FP8,
                                  kind="ExternalOutput").ap()
        dbg_encT0 = nc.dram_tensor("dbg_encT0", [128, 2, S], FP8,
                                   kind="ExternalOutput").ap()

    n_layers = reps * L

    with tile.TileContext(nc) as tc:
        with (
            tc.tile_pool(name="state", bufs=1) as state,
            tc.tile_pool(name="consts", bufs=1) as consts,
            tc.tile_pool(name="wpool", bufs=2) as wpool,
        ):
            # fp8 enc ping-pong: layer l reads enc8[l%2], writes enc8[(l+1)%2]
            enc8 = [state.tile([128, NT, DC], FP8, tag=f"enc8{x}",
                               name=f"enc8{x}") for x in range(2)]
            # encT pair tiles: [d-pair jp][:, j&1, s]  (fp8, transposed enc)
            encT = [state.tile([128, 2, S], FP8, tag=f"encT{jp}",
                               name=f"encT{jp}") for jp in range(NP // 2)]
            # exp'd scores: pair tp covers key tiles (2tp, 2tp+1); dim1 is
            # the DoubleRow k-subtile (even/odd t); dim2 queries from 256tp
            expT = [state.tile([128, 2, S - 256 * tp], FP8, tag=f"e{tp}",
                               name=f"expT{tp}") for tp in range(NP)]
            # landed q/k (dqk on partitions): [qT 0:S | kT S:2S]
            qkT = state.tile([128, 2 * S], BF16, tag="qkT", name="qkT")
            # local q/k partials awaiting exchange: [:, h, 0:1024]=qT half,
            # [:, h, 1024:2048]=kT half
            carryT = state.tile([128, 2, S], FP8, tag="carryT", name="carryT")
            ones8 = state.tile([128, 2, 2], FP8, tag="ones8", name="ones8")
            # persistent AG landing staging: [half][device partial]
            lnd_st = [[state.tile([128, S], FP8, tag=f"lnd{h}{d}",
                                  name=f"lnd{h}{d}") for d in range(2)]
                      for h in range(2)]

            ident8 = consts.tile([128, 128], FP8, tag="ident8")
            utm_add = consts.tile([128, 128], F32, tag="utm_add")
            nc.sync.dma_start(ident8[:], ident8_in)
            nc.sync.dma_start(utm_add[:], utm_add_in)

            nc.vector.memset(ones8[:], 1.0)
            for tp in range(NP):
                nc.vector.memset(expT[tp][:, 1, 0:128], 0.0)

            def emit_ag_half(h):
                dst = cc_in_all[:, h, :] if one_ag else cc_in_h[h]
                for q in range(2):
                    nc.sync.dma_start(
                        dst[:, q * 1024:(q + 1) * 1024],
                        carryT[:, h, q * 1024:(q + 1) * 1024])
                if no_ag:
                    return
                if one_ag:
                    if h == 0:  # both halves staged; gather once
                        nc.gpsimd.collective_compute(
                            "AllGather",
                            mybir.AluOpType.bypass,
                            replica_groups=GROUPS,
                            ins=[cc_in_all],
                            outs=[cc_out_all],
                        )
                else:
                    nc.gpsimd.collective_compute(
                        "AllGather",
                        mybir.AluOpType.bypass,
                        replica_groups=GROUPS,
                        ins=[cc_in_h[h]],
                        outs=[cc_out_h[h]],
                    )
                if not one_ag:
                    for d in range(2):
                        src_ = cc_in_h[h] if no_ag else cc_out_h[h][d]
                        for q in range(2):
                            nc.sync.dma_start(
                                lnd_st[h][d][:, q * 1024:(q + 1) * 1024],
                                src_[:, q * 1024:(q + 1) * 1024])

            def phase_half(lg, h, w_tile, nxt):
                """Transposes + projections for s-rows [1024h, 1024h+1024)
                of enc8[nxt], writing carryT[:, h, :]; then AG half h."""
                with tc.tile_pool(name=f"trps{lg}_{h}", bufs=2,
                                  space="PSUM") as trps:
                    for j in range(NDT):
                        ptr = trps.tile([128, 2048], FP8, tag="tr",
                                        name=f"tr{lg}_{h}_{j}")
                        for m in reversed(range(8)):
                            u = 8 * h + m
                            nc.tensor.transpose(
                                ptr[:, m * 256: m * 256 + 256: 2],
                                enc8[nxt][:, u, j * 128:(j + 1) * 128],
                                ident8[:],
                            )
                        dst = encT[j >> 1][:, j & 1,
                                           1024 * h: 1024 * h + 1024]
                        if j % 2 == 0:
                            nc.vector.tensor_copy(dst, ptr[:, 0:2048:2])
                        else:
                            nc.scalar.activation(dst, ptr[:, 0:2048:2], Copy)
                with tc.tile_pool(name=f"pjps{lg}_{h}", bufs=2,
                                  space="PSUM") as pjps:
                    for mh in range(2):   # 0: q, 1: k
                        for c in range(4):
                            pj = pjps.tile([128, 256], F32, tag="pj",
                                           name=f"pj{lg}_{h}_{mh}_{c}")
                            sl = slice(1024 * h + c * 256,
                                       1024 * h + c * 256 + 256)
                            for jp in range(NP // 2):
                                nc.tensor.matmul(
                                    pj[:],
                                    w_tile[:, 2 * jp: 2 * jp + 2,
                                           mh * 128: mh * 128 + 128],
                                    encT[jp][:, :, sl],
                                    start=(jp == 0), stop=(jp == 3),
                                    perf_mode=DR,
                                )
                            dst = carryT[:, h, 1024 * mh + c * 256:
                                         1024 * mh + c * 256 + 256]
                            if mh == 0:
                                nc.scalar.activation(dst, pj[:], Copy)
                            else:
                                nc.vector.tensor_copy(dst, pj[:])
                if not skip_cc:
                    emit_ag_half(h)

            # ---- build enc0 -> enc8[0] ----
            with tc.tile_pool(name="tmp0", bufs=1) as tmp0:
                tok = tmp0.tile([128, NT], F32, tag="tok", name="tok")
                iota = tmp0.tile([128, DC], F32, tag="iota", name="iota")
                nc.sync.dma_start(tok[:], tok_col)
                nc.sync.dma_start(iota[:], iota_nat)
                for u in reversed(range(NT)):
                    eng = nc.vector if u % 2 == 0 else nc.gpsimd
                    tmp = tmp0.tile([128, DC], F32R, tag=f"t{u % 4}",
                                    name=f"tmp{u}")
                    nc.sync.dma_start(tmp[:], pe_nat[u * 128:(u + 1) * 128, :])
                    oh = tmp0.tile([128, DC], F32, tag=f"oh{u % 4}",
                                   name=f"oh{u}")
                    eng.tensor_scalar(
                        oh[:], iota[:], tok[:, u:u + 1], None,
                        mybir.AluOpType.is_equal,
                    )
                    eng.tensor_tensor(tmp[:], tmp[:], oh[:], Add)
                    nc.scalar.activation(enc8[0][:, u, :], tmp[:], Copy)

                # initial q/k projections for layer 0 + both AG halves
                w0 = wpool.tile([128, NDT, 256], FP8, tag="w", name="w_init")
                nc.sync.dma_start(w0[:], w_stk_in[0])
                phase_half(-1, 1, w0, 0)
                phase_half(-1, 0, w0, 0)
                if dbg:
                    nc.sync.dma_start(dbg_enc8, enc8[0][:])
                    nc.sync.dma_start(dbg_carry, carryT[:])
                    nc.sync.dma_start(dbg_encT0, encT[0][:])

            # ---- layers ----
            for lg in range(n_layers):
                l0 = lg % L
                cur, nxt = lg % 2, (lg + 1) % 2
                last = lg == n_layers - 1

                if not last:
                    w_next = wpool.tile([128, NDT, 256], FP8, tag="w",
                                        name=f"w{lg}")
                    nc.sync.dma_start(w_next[:], w_stk_in[(lg + 1) % L])

                # == S1: land AG halves, scores, exp ==
                with (
                    tc.tile_pool(name=f"land{lg}", bufs=2) as landp,
                    tc.tile_pool(name=f"scps{lg}", bufs=2,
                                 space="PSUM") as scps,
                ):
                    for h in (1, 0):
                        if skip_cc:
                            nc.vector.tensor_copy(
                                qkT[:, 1024 * h: 1024 * h + 1024],
                                carryT[:, h, 0:1024])
                            nc.gpsimd.tensor_copy(
                                qkT[:, S + 1024 * h: S + 1024 * h + 1024],
                                carryT[:, h, 1024:2048])
                        else:
                            if one_ag:
                                lnd = [landp.tile([128, S], FP8,
                                                  tag=f"lnd{d}",
                                                  name=f"lnd{lg}_{h}_{d}")
                                       for d in range(2)]
                                for d in range(2):
                                    src = (cc_in_all[:, h, :] if no_ag
                                           else cc_out_all[d][:, h, :])
                                    for q in range(2):
                                        nc.sync.dma_start(
                                            lnd[d][:, q * 1024:
                                                   (q + 1) * 1024],
                                            src[:, q * 1024:(q + 1) * 1024])
                            else:
                                lnd = lnd_st[h]
                            nc.vector.tensor_tensor(
                                qkT[:, 1024 * h: 1024 * h + 1024],
                                lnd[0][:, 0:1024], lnd[1][:, 0:1024], Add)
                            nc.gpsimd.tensor_tensor(
                                qkT[:, S + 1024 * h: S + 1024 * h + 1024],
                                lnd[0][:, 1024:2048], lnd[1][:, 1024:2048],
                                Add)
                        for t in reversed(range(8 * h, 8 * h + 8)):
                            tp, row = t >> 1, t & 1
                            nt_cols = S - 128 * t
                            kT_t = qkT[:, S + t * 128: S + (t + 1) * 128]
                            for base in range(0, nt_cols, 1024):
                                wc = min(1024, nt_cols - base)
                                psc = scps.tile([128, 1024], F32, tag="sc",
                                                name=f"sc{lg}_{t}_{base}")
                                for ch in range((wc + 511) // 512):
                                    w = min(512, wc - ch * 512)
                                    off = base + ch * 512
                                    nc.tensor.matmul(
                                        psc[:, ch * 512: ch * 512 + w],
                                        kT_t,
                                        qkT[:, 128 * t + off:
                                            128 * t + off + w],
                                        start=True, stop=True,
                                    )
                                if base == 0:
                                    nc.vector.tensor_tensor(
                                        psc[:, 0:128], psc[:, 0:128],
                                        utm_add[:], Add)
                                nc.scalar.activation(
                                    expT[tp][:, row,
                                             128 * row + base:
                                             128 * row + base + wc],
                                    psc[:, 0:wc], Exp, scale=SCALE,
                                )

                if dbg and lg == 0:
                    nc.sync.dma_start(dbg_qkT, qkT[:])
                    for tp in range(NP):
                        nc.sync.dma_start(
                            dbg_expT0[tp][:, :, 0:S - 256 * tp], expT[tp][:])

                # == S2 (+ interleaved next-layer phases) ==
                with (
                    tc.tile_pool(name=f"ops{lg}", bufs=2,
                                 space="PSUM") as ops,
                    tc.tile_pool(name=f"dnps{lg}", bufs=2,
                                 space="PSUM") as dnps,
                    tc.tile_pool(name=f"rc{lg}", bufs=2) as rcp,
                    tc.tile_pool(name=f"stg{lg}", bufs=2) as stg,
                ):
                    for i in range(NT - 1, -1, -1):
                        pso = ops.tile([128, DC], F32, tag="o",
                                       name=f"o{lg}_{i}")
                        dn = dnps.tile([128, 2], F32, tag="d",
                                       name=f"d{lg}_{i}")
                        # descending pair order matches exp emission order
                        # (strips t=15..8 then 7..0), minimizing stalls
                        tps = (list(range(i // 2, 3, -1))
                               + list(range(3, -1, -1))
                               if i >= 8 else list(range(i // 2, -1, -1)))
                        first_tp, last_tp = tps[0], tps[-1]
                        if no_dr:
                            ts_or = [2 * tp + r for tp in tps for r in (0, 1)
                                     if 2 * tp + r <= i]
                            for t in ts_or:
                                tp, row = t >> 1, t & 1
                                blk1 = expT[tp][:, row, (i - 2 * tp) * 128:
                                                (i - 2 * tp) * 128 + 128]
                                for c in range(2):
                                    sl = slice(c * 512, (c + 1) * 512)
                                    nc.tensor.matmul(
                                        pso[:, sl], blk1,
                                        enc8[cur][:, t, sl],
                                        start=(t == ts_or[0]),
                                        stop=(t == ts_or[-1]),
                                    )
                                if not no_dn:
                                    nc.tensor.matmul(
                                        dn[:], blk1, ones8[:, 0, :],
                                        start=(t == ts_or[0]),
                                        stop=(t == ts_or[-1]),
                                    )
                        else:
                          for tp in tps:
                            blk = expT[tp][:, :, (i - 2 * tp) * 128:
                                           (i - 2 * tp) * 128 + 128]
                            for c in range(4):
                                sl = slice(c * 256, (c + 1) * 256)
                                # one accumulation group per 2KB PSUM bank:
                                # start only on the bank's first write
                                # (start clears accumulate-bits bank-wide)
                                nc.tensor.matmul(
                                    pso[:, sl], blk,
                                    enc8[cur][:, 2 * tp: 2 * tp + 2, sl],
                                    start=(tp == first_tp and c % 2 == 0),
                                    stop=(tp == last_tp),
                                    perf_mode=DR,
                                    skip_group_check=True,
                                )
                            if not no_dn:
                                nc.tensor.matmul(
                                    dn[:], blk, ones8[:],
                                    start=(tp == first_tp),
                                    stop=(tp == last_tp),
                                    perf_mode=DR,
                                )
                        rec = rcp.tile([128, 1], F32, tag="r",
                                       name=f"r{lg}_{i}")
                        if no_dn:
                            nc.vector.memset(rec[:], 1.0)
                        else:
                            nc.vector.reciprocal(rec[:], dn[:, 0:1])
                        if dbg and lg == 0:
                            dnsb = rcp.tile([128, 1], F32, tag="dnsb",
                                            name=f"dnsb{i}")
                            nc.vector.tensor_copy(dnsb[:], dn[:, 0:1])
                            nc.sync.dma_start(dbg_dn[i], dnsb[:])
                        if not last:
                            if i % 2 == 0:
                                nc.scalar.activation(
                                    enc8[nxt][:, i, :], pso[:], Copy,
                                    scale=rec[:])
                            else:
                                nc.vector.tensor_scalar(
                                    enc8[nxt][:, i, :], pso[:], rec[:],
                                    None, Mult)
                        if last:
                            stage = stg.tile([128, DC], F32R, tag="s",
                                             name=f"stg{lg}_{i}")
                            if i % 2 == 0:
                                nc.scalar.activation(
                                    stage[:], pso[:], Copy, scale=rec[:])
                            else:
                                nc.vector.tensor_scalar(
                                    stage[:], pso[:], rec[:], None, Mult)
                            nc.sync.dma_start(
                                out_dram[i * 128:(i + 1) * 128, :], stage[:])
                        if not last and not no_phase:
                            if i == 8:
                                phase_half(lg, 1, w_next, nxt)
                            elif i == 0:
                                phase_half(lg, 0, w_next, nxt)
                    if dbg and lg == 0:
                        nc.sync.dma_start(dbg_enc1, enc8[1][:])

    nc.compile()
    return nc


def _pe_table():
    pos = np.arange(S, dtype=np.float32)[:, None]
    half = np.arange(0, D, 2, dtype=np.float32)
    div = np.exp(-(np.log(np.float32(10000.0)) / np.float32(D)) * half)
    pe = np.zeros((S, D), np.float32)
    pe[:, 0::2] = np.sin(pos * div)
    pe[:, 1::2] = np.cos(pos * div)
    return pe


def _prepare_in_maps(token_ids, Wq, Wk):
    import ml_dtypes

    E4 = ml_dtypes.float8_e4m3

    token_ids = np.asarray(token_ids)
    Wq = np.asarray(Wq, dtype=np.float32) * WSCALE
    Wk = np.asarray(Wk, dtype=np.float32) * WSCALE

    pe = _pe_table()
    ident8 = np.eye(128).astype(E4)
    utm_add = (np.tril(np.ones((128, 128), np.float32), -1)
               * np.float32(-1e9))

    # w_stk[l, p, j, 0:128] = 16*Wq[l, c*DC + 128j + p, :]; [128:256] = Wk
    def w_stack(c):
        out = np.empty((L, 128, NDT, 256), np.float32)
        for j in range(NDT):
            rows = slice(c * DC + j * 128, c * DC + (j + 1) * 128)
            out[:, :, j, 0:128] = Wq[:, rows, :]
            out[:, :, j, 128:256] = Wk[:, rows, :]
        return out.astype(E4)

    in_maps = []
    for core in range(N_CORES):
        b, c = divmod(core, SPLIT)
        toks = token_ids[b % B].astype(np.float32)
        tok_col = np.ascontiguousarray(toks.reshape(NT, 128).T)  # [128, NT]
        iota_nat = np.broadcast_to(
            (np.arange(DC, dtype=np.float32) + c * DC)[None, :], (128, DC)
        ).copy()
        in_maps.append({
            "pe_nat": np.ascontiguousarray(pe[:, c * DC:(c + 1) * DC]),
            "tok_col": tok_col,
            "iota_nat": iota_nat,
            "w_stk": w_stack(c),
            "ident8": ident8,
            "utm_add": utm_add,
        })
    return in_maps


def kernel(token_ids, Wq, Wk, _trace=False):
    from concourse.bass_utils import run_bass_kernel_spmd

    if "nc" not in _CACHED:
        _CACHED["nc"] = _build()
    nc = _CACHED["nc"]

    import hashlib
    key = hashlib.md5(
        np.asarray(token_ids).tobytes() + np.asarray(Wq).tobytes()
        + np.asarray(Wk).tobytes()
    ).hexdigest()
    if _CACHED.get("in_key") != key:
        _CACHED["in_maps"] = _prepare_in_maps(token_ids, Wq, Wk)
        _CACHED["in_key"] = key

    try:
        res = run_bass_kernel_spmd(
            nc, _CACHED["in_maps"], core_ids=list(range(N_CORES)), trace=_trace,
        )
    except Exception:
        # axon tunnel occasionally drops a worker transiently; retry once
        res = run_bass_kernel_spmd(
            nc, _CACHED["in_maps"], core_ids=list(range(N_CORES)), trace=_trace,
        )
    _CACHED["last_result"] = res

    out = np.empty((B, S, D), np.float32)
    for core in range(N_CORES):
        b, c = divmod(core, SPLIT)
        out[b][:, c * DC:(c + 1) * DC] = res.results[core]["out"]
    return out
